# revision 2
# baseline (speedup 1.0000x reference)
"""Trainium2 Bass kernel for GPT-2-style causal multi-head attention.

Problem: x[2, 2048, 1024]; qkv = x@w_attn+b_attn; 16 heads, head_dim 64;
causal softmax((q k^T)/8) @ v; out = merge @ w_proj + b_proj.

Sharding over 8 NeuronCores: data-parallel on batch (2) x tensor-parallel
on heads (4 groups of 4 heads). Each core computes, for its (batch, head
group): the q/k/v projections restricted to its head columns, causal
attention for its 4 heads, and a partial output projection through its
256 rows of w_proj. The host sums the 4 partials per batch and adds b_proj.

On-device layout (per core):
  - x^T [1024, 2048] streamed in (host pre-transposes + casts bf16)
  - q^T, k^T computed as [d, s] (w_attn cols stationary, x^T moving)
  - v computed as [s, d] (x^T stationary, w_v moving), with an extra
    all-ones column interleaved per head so the attention A@[V|1] matmul
    also produces the softmax denominator row.
  - scores^T [k, s_q] per head pair; 2 concurrent K=64 matmuls via PE
    row tiling (partitions 0:64 / 64:128). Causal: only k-tiles with
    k0 < q0+512 are computed; diagonal 128x128 blocks get an additive
    -1e9 mask; exp(scale*x) on ScalarE; A@V accumulates over k in PSUM.
  - normalization: reciprocal of the denominator row, broadcast across
    partitions via a DRAM bounce DMA, multiplied into a^T during the
    PSUM->SBUF eviction.
  - partial out^T [1024, 2048] = w_proj_rows^T stationary @ a^T moving.
"""

import sys

sys.path.insert(0, "/opt/trn_rl_repo")

import numpy as np
import ml_dtypes

BF16 = ml_dtypes.bfloat16

NX = 1024
N_HEAD = 16
HEAD_DIM = 64
S = 2048
B = 2
N_CORES = 8
HPC = 4  # heads per core

_CACHE = {}


def build_nc(n_iters=1):
    import concourse.bass as bass
    import concourse.bacc as bacc
    import concourse.tile as tile
    import concourse.mybir as mybir

    f32 = mybir.dt.float32
    bf = mybir.dt.bfloat16
    EXP = mybir.ActivationFunctionType.Exp

    nc = bacc.Bacc(trn_type="TRN2")
    xT = nc.dram_tensor("xT", [NX, S], bf, kind="ExternalInput")
    wqk = nc.dram_tensor("wqk", [NX, 512], bf, kind="ExternalInput")
    bqk = nc.dram_tensor("bqk", [1, 512], bf, kind="ExternalInput")
    wv = nc.dram_tensor("wv", [NX, 260], bf, kind="ExternalInput")
    bv = nc.dram_tensor("bv", [1, 260], bf, kind="ExternalInput")
    wp = nc.dram_tensor("wp", [256, NX], bf, kind="ExternalInput")
    mask = nc.dram_tensor("mask", [128, 128], f32, kind="ExternalInput")
    po = nc.dram_tensor("po", [NX, S], f32, kind="ExternalOutput")

    def body(tc, ctx):
        sb = ctx.enter_context(tc.tile_pool(name="sb", bufs=1))
        wk = ctx.enter_context(tc.tile_pool(name="wk", bufs=6))
        ps = ctx.enter_context(tc.tile_pool(name="ps", bufs=2, space="PSUM"))
        dr = ctx.enter_context(tc.tile_pool(name="dr", bufs=4, space="DRAM"))

        xT_sb = [sb.tile([128, S], bf, tag=f"xT{i}", name=f"xT{i}") for i in range(8)]
        wqk_sb = [sb.tile([128, 512], bf, tag=f"wqk{i}", name=f"wqk{i}") for i in range(8)]
        wv_sb = [sb.tile([128, 260], bf, tag=f"wv{i}", name=f"wv{i}") for i in range(8)]
        wp_sb = [sb.tile([128, NX], bf, tag=f"wp{m}", name=f"wp{m}") for m in range(2)]
        bqk_sb = sb.tile([1, 512], bf, tag="bqk")
        bv_sb = sb.tile([1, 260], bf, tag="bv")
        ones_sb = sb.tile([1, 512], bf, tag="ones")
        mask_sb = sb.tile([128, 128], f32, tag="mask")
        qT_sb = [sb.tile([128, S], bf, tag=f"qT{p}", name=f"qT{p}") for p in range(2)]
        kT_sb = [sb.tile([128, S], bf, tag=f"kT{p}", name=f"kT{p}") for p in range(2)]
        v_sb = [sb.tile([128, 260], bf, tag=f"v{s}", name=f"v{s}") for s in range(16)]
        aT_sb = [sb.tile([128, S], bf, tag=f"aT{p}", name=f"aT{p}") for p in range(2)]

        for i in range(8):
            nc.sync.dma_start(out=xT_sb[i][:], in_=xT[128 * i : 128 * (i + 1), :])
            nc.sync.dma_start(out=wqk_sb[i][:], in_=wqk[128 * i : 128 * (i + 1), :])
            nc.sync.dma_start(out=wv_sb[i][:], in_=wv[128 * i : 128 * (i + 1), :])
        for m in range(2):
            nc.sync.dma_start(out=wp_sb[m][:], in_=wp[128 * m : 128 * (m + 1), :])
        nc.sync.dma_start(out=bqk_sb[:], in_=bqk[:])
        nc.sync.dma_start(out=bv_sb[:], in_=bv[:])
        nc.sync.dma_start(out=mask_sb[:], in_=mask[:])
        nc.vector.memset(ones_sb[:], 1.0)

        # Phase A1: q^T, k^T = (w_qk stationary)^T @ x^T  [+ bias x ones]
        for ot in range(4):
            dst = qT_sb[ot] if ot < 2 else kT_sb[ot - 2]
            for st in range(4):
                p1 = ps.tile([128, 512], f32, tag="sA")
                for i in range(8):
                    nc.tensor.matmul(
                        p1[:],
                        lhsT=wqk_sb[i][:, 128 * ot : 128 * (ot + 1)],
                        rhs=xT_sb[i][:, 512 * st : 512 * (st + 1)],
                        start=(i == 0),
                        stop=False,
                    )
                nc.tensor.matmul(
                    p1[:],
                    lhsT=bqk_sb[0:1, 128 * ot : 128 * (ot + 1)],
                    rhs=ones_sb[:],
                    start=False,
                    stop=True,
                )
                nc.vector.tensor_copy(dst[:, 512 * st : 512 * (st + 1)], p1[:])

        # Phase A2: v = (x^T stationary)^T @ w_v  [+ ones x bias]
        for st in range(16):
            p1 = ps.tile([128, 260], f32, tag="sB")
            for i in range(8):
                nc.tensor.matmul(
                    p1[:],
                    lhsT=xT_sb[i][:, 128 * st : 128 * (st + 1)],
                    rhs=wv_sb[i][:],
                    start=(i == 0),
                    stop=False,
                )
            nc.tensor.matmul(
                p1[:],
                lhsT=ones_sb[0:1, 0:128],
                rhs=bv_sb[:],
                start=False,
                stop=True,
            )
            nc.vector.tensor_copy(v_sb[st][:], p1[:])

        # Phase B: causal attention, 2 head pairs x 4 q-tiles of 512
        for p2 in range(2):
            for qi in range(4):
                accA = ps.tile([128, 512], f32, tag="accA")
                accB = ps.tile([128, 512], f32, tag="accB")
                nkt = 4 * qi + 4
                for kt in range(nkt):
                    j = kt - 4 * qi  # >=0 -> diagonal block index
                    c0 = 128 * j if j > 0 else 0
                    ksl = slice(128 * kt, 128 * (kt + 1))
                    qsl = slice(512 * qi + c0, 512 * (qi + 1))
                    sA = ps.tile([128, 512], f32, tag="sA")
                    sB = ps.tile([128, 512], f32, tag="sB")
                    nc.tensor.matmul(
                        sA[:, c0:512], lhsT=kT_sb[p2][0:64, ksl],
                        rhs=qT_sb[p2][0:64, qsl], start=True, stop=True,
                    )
                    nc.tensor.matmul(
                        sB[:, c0:512], lhsT=kT_sb[p2][64:128, ksl],
                        rhs=qT_sb[p2][64:128, qsl], start=True, stop=True,
                    )
                    if j >= 0:
                        blk = slice(128 * j, 128 * (j + 1))
                        nc.vector.tensor_add(sA[:, blk], sA[:, blk], mask_sb[:])
                        nc.vector.tensor_add(sB[:, blk], sB[:, blk], mask_sb[:])
                    eA = wk.tile([128, 512], bf, tag="eA")
                    eB = wk.tile([128, 512], bf, tag="eB")
                    nc.scalar.activation(eA[:, c0:512], sA[:, c0:512], EXP, scale=0.125)
                    nc.scalar.activation(eB[:, c0:512], sB[:, c0:512], EXP, scale=0.125)
                    vA = v_sb[kt][:, 65 * (2 * p2) : 65 * (2 * p2) + 65]
                    vB = v_sb[kt][:, 65 * (2 * p2 + 1) : 65 * (2 * p2 + 1) + 65]
                    nc.tensor.matmul(
                        accA[0:65, c0:512], lhsT=vA, rhs=eA[:, c0:512],
                        start=(kt == 0), stop=(kt == nkt - 1),
                    )
                    nc.tensor.matmul(
                        accB[0:65, c0:512], lhsT=vB, rhs=eB[:, c0:512],
                        start=(kt == 0), stop=(kt == nkt - 1),
                    )
                for acc, ro in ((accA, 0), (accB, 64)):
                    rc = wk.tile([1, 512], f32, tag="rc")
                    nc.vector.reciprocal(rc[:], acc[64:65, :])
                    scr = dr.tile([1, 512], f32, tag="scr")
                    nc.sync.dma_start(out=scr[:], in_=rc[:])
                    bc = wk.tile([64, 512], f32, tag="bc")
                    sap = scr[:]
                    import concourse.bass as _b

                    nc.sync.dma_start(
                        out=bc[:],
                        in_=_b.AP(
                            tensor=sap.tensor,
                            offset=sap.offset,
                            ap=[[0, 64]] + list(sap.ap)[1:],
                        ),
                    )
                    nc.vector.tensor_mul(
                        aT_sb[p2][ro : ro + 64, 512 * qi : 512 * (qi + 1)],
                        acc[0:64, :],
                        bc[:],
                    )

        # Phase C: partial out^T = (w_proj rows stationary)^T @ a^T
        for nt in range(8):
            for st in range(4):
                pC = ps.tile([128, 512], f32, tag="accA")
                for m in range(2):
                    nc.tensor.matmul(
                        pC[:],
                        lhsT=wp_sb[m][:, 128 * nt : 128 * (nt + 1)],
                        rhs=aT_sb[m][:, 512 * st : 512 * (st + 1)],
                        start=(m == 0),
                        stop=(m == 1),
                    )
                oC = wk.tile([128, 512], f32, tag="oC")
                nc.vector.tensor_copy(oC[:], pC[:])
                nc.sync.dma_start(
                    out=po[128 * nt : 128 * (nt + 1), 512 * st : 512 * (st + 1)],
                    in_=oC[:],
                )

    from contextlib import ExitStack

    with tile.TileContext(nc) as tc:
        if n_iters > 1:
            import concourse.engine_type as engine_type

            with tc.For_i(0, n_iters, 1, hint_engines=(mybir.EngineType.PE,)):
                with ExitStack() as ctx:
                    body(tc, ctx)
        else:
            with ExitStack() as ctx:
                body(tc, ctx)
    nc.finalize()
    return nc


def _make_runner(nc, n_cores=N_CORES):
    """Build the PJRT callable once (adapted from bass2jax.run_bass_via_pjrt)."""
    import jax
    import jax.core
    from jax.sharding import Mesh, PartitionSpec
    from jax.experimental.shard_map import shard_map
    import concourse.mybir as mybir
    from concourse import bass2jax

    bass2jax.install_neuronx_cc_hook()
    partition_name = nc.partition_id_tensor.name if nc.partition_id_tensor else None
    in_names, out_names, out_avals, zero_outs = [], [], [], []
    for alloc in nc.m.functions[0].allocations:
        if not isinstance(alloc, mybir.MemoryLocationSet):
            continue
        name = alloc.memorylocations[0].name
        if alloc.kind == "ExternalInput":
            if name != partition_name:
                in_names.append(name)
        elif alloc.kind == "ExternalOutput":
            shape = tuple(alloc.tensor_shape)
            dtype = mybir.dt.np(alloc.dtype)
            out_names.append(name)
            out_avals.append(jax.core.ShapedArray(shape, dtype))
            zero_outs.append(np.zeros(shape, dtype))
    n_params = len(in_names)
    all_in_names = list(in_names) + list(out_names)
    if partition_name is not None:
        all_in_names.append(partition_name)

    def _body(*args):
        operands = list(args)
        if partition_name is not None:
            operands.append(bass2jax.partition_id_tensor())
        outs = bass2jax._bass_exec_p.bind(
            *operands,
            out_avals=tuple(out_avals),
            in_names=tuple(all_in_names),
            out_names=tuple(out_names),
            lowering_input_output_aliases=(),
            sim_require_finite=True,
            sim_require_nnan=True,
            nc=nc,
        )
        return tuple(outs)

    devices = jax.devices()[:n_cores]
    mesh = Mesh(np.asarray(devices), ("core",))
    nin = n_params + len(out_names)
    sharded = jax.jit(
        shard_map(
            _body,
            mesh=mesh,
            in_specs=(PartitionSpec("core"),) * nin,
            out_specs=(PartitionSpec("core"),) * len(out_names),
            check_rep=False,
        ),
        keep_unused=True,
    )

    def prep(in_maps):
        import jax as _jax

        per_core = [[np.asarray(m[name]) for name in in_names] for m in in_maps]
        concat_in = [
            np.concatenate([per_core[c][i] for c in range(n_cores)], axis=0)
            for i in range(n_params)
        ]
        concat_zeros = [
            np.zeros((n_cores * z.shape[0], *z.shape[1:]), z.dtype) for z in zero_outs
        ]
        return [_jax.device_put(a) for a in concat_in + concat_zeros]

    def run(dev_args):
        import jax as _jax

        outs = sharded(*dev_args)
        _jax.block_until_ready(outs)
        return [
            {
                name: np.asarray(outs[i]).reshape(n_cores, *out_avals[i].shape)[c]
                for i, name in enumerate(out_names)
            }
            for c in range(n_cores)
        ]

    return prep, run


def get_runner(n_iters=1):
    key = n_iters
    if key not in _CACHE:
        nc = build_nc(n_iters)
        _CACHE[key] = _make_runner(nc)
    return _CACHE[key]


def make_in_maps(x, w_attn, b_attn, w_proj, b_proj):
    tri = np.where(
        np.arange(128)[:, None] <= np.arange(128)[None, :], 0.0, -1e9
    ).astype(np.float32)
    in_maps = []
    for c in range(N_CORES):
        b = c // 4
        g = c % 4
        qc = slice(256 * g, 256 * g + 256)
        kc = slice(NX + 256 * g, NX + 256 * g + 256)
        vc = slice(2 * NX + 256 * g, 2 * NX + 256 * g + 256)
        wqk_c = np.concatenate([w_attn[:, qc], w_attn[:, kc]], axis=1)
        bqk_c = np.concatenate([b_attn[qc], b_attn[kc]])[None, :]
        wv_c = np.zeros((NX, 260), np.float32)
        bv_c = np.zeros((1, 260), np.float32)
        wv_base = w_attn[:, vc]
        bv_base = b_attn[vc]
        for hi in range(HPC):
            wv_c[:, 65 * hi : 65 * hi + 64] = wv_base[:, 64 * hi : 64 * hi + 64]
            bv_c[0, 65 * hi : 65 * hi + 64] = bv_base[64 * hi : 64 * hi + 64]
            bv_c[0, 65 * hi + 64] = 1.0
        in_maps.append(
            {
                "xT": np.ascontiguousarray(x[b].T).astype(BF16),
                "wqk": wqk_c.astype(BF16),
                "bqk": bqk_c.astype(BF16),
                "wv": wv_c.astype(BF16),
                "bv": bv_c.astype(BF16),
                "wp": np.ascontiguousarray(w_proj[256 * g : 256 * g + 256, :]).astype(
                    BF16
                ),
                "mask": tri,
            }
        )
    return in_maps


def gather_output(results, b_proj):
    out = np.zeros((B, S, NX), np.float32)
    for c in range(N_CORES):
        out[c // 4] += results[c]["po"].T
    out += b_proj[None, None, :].astype(np.float32)
    return out


def kernel(x, w_attn, b_attn, w_proj, b_proj):
    x = np.asarray(x, np.float32)
    w_attn = np.asarray(w_attn, np.float32)
    b_attn = np.asarray(b_attn, np.float32)
    w_proj = np.asarray(w_proj, np.float32)
    b_proj = np.asarray(b_proj, np.float32)
    prep, run = get_runner(1)
    dev = prep(make_in_maps(x, w_attn, b_attn, w_proj, b_proj))
    results = run(dev)
    return gather_output(results, b_proj)
